# revision 4
# baseline (speedup 1.0000x reference)
"""DigitCapsule dynamic-routing kernel for 8 TRN2 NeuronCores.

Strategy: the reference routing is fully independent per output capsule c
(softmax over routes, sums over routes, batch-mean are all per-c). So we
shard the C=64 capsules 8-ways: each core gets W[:, 8k:8k+8] and a
replicated x. Zero collectives; identical SPMD program per core with
per-core inputs.

Per core (B=64, R=2048, I=8, CL=8, O=16; K-dim = (r,i) = 16384 = 128
k-tiles of 128 = (16 routes q, 8 i)). s/v tensors live as
[b=64, (o,c)=128]; routing state lives banded as [(j,q)=128, (g,lo,c)].

  pass 0:  n0[b,(o,c)] = sum_t xt_t^T @ wk_t    (c_ij uniform), with
           col-paired matmuls (even k-tiles -> psum rows 0:64, odd ->
           64:128; fold with one cross-base add)
           v = n|n| / (R^2 + n^2)       == squash(n/R), exact algebra
  iter 1,2:
    A: G[(q,i),(lo,(o,c))] = xn^T @ V, row-paired: two concurrent K=64
       matmuls (fp8 x halves on partitions 0:64 / 64:128) into the two
       psum banks of each block; per block the P = G (.) Wk multiply
       runs on DVE (direct from psum) or ACT-drain + DVE/GPS
    B: BD-matmul bands psb[(j,q),(lo,o,c)] per grp (interleaved with A's
       G-matmuls so the PE never idles); ored = reduce_o; bstate +=
       ored/B; wexpb = exp(bstate); wrep matmuls merged 4-per-psum-tile
    D: WW = Wr (.) wrep (broadcast o) on DVE/GPS; n += xt_t^T @ WW_t
       col-paired
    Z[c] = sum_r wexp;  v = n|n| / (Z^2 + n^2)  == squash(n/Z), exact
  out[b,(o,c)] = v (f32)

V is duplicated into both partition halves of Vz via a small replication
matmul (I2) so the row-paired G matmuls can read per-half rhs.
"""

import os
import sys

for _p in ("/opt/trn_rl_repo", "/root/.axon_site/_ro/trn_rl_repo"):
    if os.path.isdir(_p) and _p not in sys.path:
        sys.path.insert(0, _p)

from contextlib import ExitStack

import numpy as np

import concourse.bass as bass
import concourse.bacc as bacc
from concourse import mybir
from concourse.bass_utils import run_bass_kernel_spmd
from concourse.tile import TileContext

B, R, C, O, I = 64, 2048, 64, 16, 8
N_CORES = 8
CL = C // N_CORES            # capsules per core = 8
F = CL * O                   # free (o,c) = 128
NT = R // 16                 # 128 k-tiles; tile t = routes [16t,16t+16), part p=(q,i)
NB = 16                      # number of 8-k-tile blocks
BLK = NT // NB               # 8 k-tiles per block

# P-production path per block: DVE-direct from psum / ACT drain + DVE /
# ACT drain + GPS
DIRECT_SET = {0, 2, 5, 8, 10, 13}
P_GPS_SET = {3, 7, 11, 15}
# WW multiply engine per block (GPS for these, DVE otherwise)
WW_GPS_SET = {2, 5, 8, 11}


def _consts_np():
    """cstb [128,1152] bf16: BDF4 [0:512), BDT [512:1024), I2 dup [1024:1152).
    cstf [128,65] f32: masked-ones col 0; ones-row (partition 0) cols [1:65)."""
    cstb = np.zeros((128, 1152), dtype=np.float32)
    p = np.arange(128)
    # BDF4_j[p=(q,i), m] = 1 iff m == 32j + p//8  (i-reduce into band 32j+q)
    for j in range(4):
        cstb[p, 128 * j + 32 * j + p // 8] = 1.0
    # BDT_j = BDF4_j^T (band (j,q) -> rows (q,i))
    for j in range(4):
        cstb[:, 512 + 128 * j:512 + 128 * (j + 1)] = \
            cstb[:, 128 * j:128 * (j + 1)].T
    # I2[b, m] = 1 iff m % 64 == b (for b < 64): psum dup of V to both halves
    cstb[p[:64], 1024 + p[:64]] = 1.0
    cstb[p[:64], 1024 + 64 + p[:64]] = 1.0
    cstf = np.zeros((128, 65), dtype=np.float32)
    # Z-reduce mask: only band rows 32j+q (q<16) hold real data; the other
    # 64 partitions of wexpb are exp(0)=1 junk and must not enter Z.
    cstf[p[(p % 32) < 16], 0] = 1.0
    cstf[0, 1:65] = 1.0
    return cstb, cstf


def build_bass():
    f32 = mybir.dt.float32
    cdt = mybir.dt.bfloat16
    f8 = mybir.dt.float8e4

    nc = bacc.Bacc()
    # wk: 8 chunks of 2048 cols of W (bf16), k-tile t at chunk t//16
    wk_d = [nc.declare_dram_parameter(f"wk{h}", [128, 2048], cdt, isOutput=False)
            for h in range(8)]
    # xn8p: fp8 x, pair-packed: rows 0:64 = tiles hb*8+{0..3}, rows 64:128
    # = tiles hb*8+{4..7} (each block hb owns 512 cols)
    xn8p_d = nc.declare_dram_parameter("xn8p", [128, NB * 512], f8, isOutput=False)
    # xt8: fp8 x in (q,i)-partition layout, tile t at cols [64t, 64t+64)
    xt8_d = nc.declare_dram_parameter("xt8", [128, NT * 64], f8, isOutput=False)
    # xtb: bf16 x, same layout; loaded mid-kernel for the final n-pass
    xtb_d = nc.declare_dram_parameter("xtb", [128, NT * 64], cdt, isOutput=False)
    cstb_d = nc.declare_dram_parameter("cstb", [128, 1152], cdt, isOutput=False)
    cstf_d = nc.declare_dram_parameter("cstf", [128, 65], f32, isOutput=False)
    out_d = nc.declare_dram_parameter("out", [B, F], f32, isOutput=True)

    with TileContext(nc) as tc, ExitStack() as ctx:
        big = ctx.enter_context(tc.tile_pool(name="big", bufs=1))
        small = ctx.enter_context(tc.tile_pool(name="small", bufs=3))
        pgpool = ctx.enter_context(tc.tile_pool(name="pgpool", bufs=3))
        p16 = ctx.enter_context(tc.tile_pool(name="p16", bufs=NB + 1))
        wwpool = ctx.enter_context(tc.tile_pool(name="wwpool", bufs=4))
        ps_acc = ctx.enter_context(tc.tile_pool(name="ps_acc", bufs=1, space="PSUM"))
        ps_gb = ctx.enter_context(tc.tile_pool(name="ps_gb", bufs=3, space="PSUM"))
        ps_misc = ctx.enter_context(tc.tile_pool(name="ps_misc", bufs=1, space="PSUM"))

        # ---- load inputs: many small pieces round-robined across the
        # sync/gpsimd/scalar issue queues (per-stream DMA bw is low; the
        # aggregate needs ~20 concurrent streams). Priority order: consts,
        # xt8 + first wk chunks (pass0 critical path), xn8p, rest of wk,
        # then the bf16 xtb which is only needed in iter 2's n-pass.
        cstb = big.tile([128, 1152], cdt, tag="cstb", name="cstb")
        cstf = big.tile([128, 65], f32, tag="cstf", name="cstf")
        xt8 = big.tile([128, NT * 64], f8, tag="xt8", name="xt8")
        wk = [big.tile([128, 2048], cdt, tag=f"wk{h}", name=f"wk{h}")
              for h in range(8)]
        xn8p = big.tile([128, NB * 512], f8, tag="xn8p", name="xn8p")
        xtb = big.tile([128, NT * 64], cdt, tag="xtb", name="xtb")
        dma_q = [nc.sync, nc.gpsimd, nc.scalar]
        jobs = [(cstb, cstb_d[:]), (cstf, cstf_d[:])]
        for piece in range(4):
            c0 = piece * 2048
            jobs.append((xt8[:, c0:c0 + 2048], xt8_d[:, c0:c0 + 2048]))
        for h in range(4):
            for piece in range(2):
                c0 = piece * 1024
                jobs.append((wk[h][:, c0:c0 + 1024], wk_d[h][:, c0:c0 + 1024]))
        for piece in range(4):
            c0 = piece * 2048
            jobs.append((xn8p[:, c0:c0 + 2048], xn8p_d[:, c0:c0 + 2048]))
        for h in range(4, 8):
            for piece in range(2):
                c0 = piece * 1024
                jobs.append((wk[h][:, c0:c0 + 1024], wk_d[h][:, c0:c0 + 1024]))
        for piece in range(4):
            c0 = piece * 2048
            jobs.append((xtb[:, c0:c0 + 2048], xtb_d[:, c0:c0 + 2048]))
        for idx, (dst, srcp) in enumerate(jobs):
            dma_q[idx % 3].dma_start(out=dst, in_=srcp)

        BDF4 = cstb[:, 0:512]
        BDT = cstb[:, 512:1024]
        I2 = cstb[0:64, 1024:1152]
        onesm = cstf[:, 0:1]
        onesrow = cstf[0:1, 1:65]

        def wk_tile(t):
            h, lo = t // 16, t % 16
            return wk[h][:, lo * 128:(lo + 1) * 128]

        def wk_block(hb):
            # [128, 8, 128] view of block hb's 8 k-tiles of W
            wkh = wk[hb // 2].rearrange("p (u f) -> p u f", f=128)
            return wkh[:, (hb % 2) * BLK:(hb % 2) * BLK + BLK, :]

        # V: [128,128] bf16; squash writes rows 0:64, dup-matmul fills 64:128
        Vz = big.tile([128, 128], cdt, tag="Vz", name="Vz")

        # v = n*|n| / (zsq + n^2); nf is [64,128] f32 in SBUF
        def squash_from(nf, zsq_sb, mk_V):
            absn = small.tile([64, 128], f32, tag="absn", name="absn")
            nc.scalar.activation(absn, nf, mybir.ActivationFunctionType.Abs)
            nsq = small.tile([64, 128], f32, tag="nsq", name="nsq")
            nc.scalar.activation(nsq, nf, mybir.ActivationFunctionType.Square)
            den = small.tile([64, 128], f32, tag="den", name="den")
            if zsq_sb is None:
                nc.vector.tensor_scalar_add(den, nsq, float(R) * float(R))
            else:
                nc.vector.tensor_add(den, nsq, zsq_sb)
            rden = small.tile([64, 128], f32, tag="rden", name="rden")
            nc.vector.reciprocal_approx_fast(rden, den)
            num = small.tile([64, 128], f32, tag="num", name="num")
            nc.vector.tensor_mul(num, nf, absn)
            if not mk_V:
                out_sb = small.tile([64, 128], f32, tag="outsb", name="outsb")
                nc.vector.tensor_mul(out_sb, num, rden)
                return out_sb
            nc.vector.tensor_mul(Vz[0:64, :], num, rden)
            # duplicate V into rows 64:128 via replication matmul
            ps_dup = ps_misc.tile([128, 128], f32, tag="m", name="dup")
            nc.tensor.matmul(ps_dup, lhsT=I2, rhs=Vz[0:64, :],
                             start=True, stop=True)
            nc.scalar.activation(Vz[64:128, :], ps_dup[64:128, :],
                                 mybir.ActivationFunctionType.Copy)
            return None

        # fold the two col-pair accumulator halves and return n as f32 SBUF
        def fold_n(ps_n):
            nhi = small.tile([64, 128], f32, tag="nhi", name="nhi")
            nc.scalar.activation(nhi, ps_n[64:128, :],
                                 mybir.ActivationFunctionType.Copy)
            nf = small.tile([64, 128], f32, tag="nf", name="nf")
            nc.vector.tensor_add(nf, ps_n[0:64, :], nhi)
            return nf

        # ---- pass 0: n0 = sum_t xt8_t^T @ wk_t (col-paired) ; V = squash ----
        ps_s = ps_acc.tile([128, 128], f32, tag="acc", name="acc")
        for t in range(NT):
            half = t % 2
            nc.tensor.matmul(ps_s[half * 64:(half + 1) * 64, :],
                             lhsT=xt8[:, t * 64:(t + 1) * 64],
                             rhs=wk_tile(t),
                             start=(t < 2), stop=(t >= NT - 2))
        squash_from(fold_n(ps_s), None, True)

        bstate = small.tile([128, 256], f32, tag="bstate", name="bstate", bufs=1)
        nc.vector.memset(bstate, 0.0)
        wexpb = small.tile([128, 256], cdt, tag="wexpb", name="wexpb", bufs=1)

        for it in (1, 2):
            ps_n = ps_acc.tile([128, 128], f32, tag="acc", name="acc")
            Ps = [None] * NB
            psbs = [None] * 4
            wrs = [None] * NB

            # -- phase A pieces: G row-pairs + P production for one block --
            def emit_g_block(hb):
                psg = ps_gb.tile([128, BLK * 128], f32, tag="gb", name="gb")
                for u in range(4):
                    cs = slice(hb * 512 + u * 128, hb * 512 + (u + 1) * 128)
                    nc.tensor.matmul(psg[:, u * 128:(u + 1) * 128],
                                     lhsT=xn8p[0:64, cs], rhs=Vz[0:64, :],
                                     start=True, stop=True)
                    nc.tensor.matmul(psg[:, 512 + u * 128:512 + (u + 1) * 128],
                                     lhsT=xn8p[64:128, cs], rhs=Vz[64:128, :],
                                     start=True, stop=True)
                P = p16.tile([128, BLK * 128], cdt, tag="P", name="P")
                if hb in DIRECT_SET:
                    nc.vector.tensor_tensor(
                        P.rearrange("p (u f) -> p u f", f=128),
                        psg.rearrange("p (u f) -> p u f", f=128),
                        wk_block(hb),
                        op=mybir.AluOpType.mult,
                    )
                else:
                    Pg = pgpool.tile([128, BLK * 128], cdt, tag="Pg", name="Pg")
                    nc.scalar.activation(Pg, psg,
                                         mybir.ActivationFunctionType.Copy)
                    eng = nc.gpsimd if hb in P_GPS_SET else nc.vector
                    eng.tensor_tensor(
                        P.rearrange("p (u f) -> p u f", f=128),
                        Pg.rearrange("p (u f) -> p u f", f=128),
                        wk_block(hb),
                        op=mybir.AluOpType.mult,
                    )
                Ps[hb] = P

            # -- phase B pieces --
            def emit_bd(grp):
                psb = ps_gb.tile([128, BLK * 128], f32, tag="gb", name="gb")
                for j in range(4):
                    for half in range(2):
                        nc.tensor.matmul(
                            psb[:, half * 512:(half + 1) * 512],
                            lhsT=BDF4[:, 128 * j:128 * (j + 1)],
                            rhs=Ps[4 * grp + j][:, half * 512:(half + 1) * 512],
                            start=(j == 0), stop=(j == 3),
                        )
                psbs[grp] = psb

            def emit_bupdate(grp):
                ored = small.tile([128, 64], f32, tag="ored", name="ored",
                                  bufs=2)
                psb = psbs[grp]
                nc.vector.tensor_reduce(
                    ored.rearrange("p (l c) -> p l c", c=8),
                    bass.AP(tensor=psb.tensor, offset=psb.offset,
                            ap=[psb.ap[0], [128, 8], [1, 8], [8, 16]]),
                    axis=mybir.AxisListType.X,
                    op=mybir.AluOpType.add,
                )
                cs = slice(grp * 64, (grp + 1) * 64)
                nc.vector.scalar_tensor_tensor(bstate[:, cs], ored, 1.0 / B,
                                               bstate[:, cs],
                                               op0=mybir.AluOpType.mult,
                                               op1=mybir.AluOpType.add)
                nc.scalar.activation(wexpb[:, cs], bstate[:, cs],
                                     mybir.ActivationFunctionType.Exp)

            def emit_wrep(grp):
                cs = slice(grp * 64, (grp + 1) * 64)
                ps_wr = ps_misc.tile([128, 256], f32, tag="m", name="wrps")
                for j in range(4):
                    nc.tensor.matmul(ps_wr[:, j * 64:(j + 1) * 64],
                                     lhsT=BDT[:, 128 * j:128 * (j + 1)],
                                     rhs=wexpb[:, cs], start=True, stop=True)
                wr4 = small.tile([128, 256], cdt, tag="wr", name="wr", bufs=2)
                nc.scalar.activation(wr4, ps_wr,
                                     mybir.ActivationFunctionType.Copy)
                for j in range(4):
                    wrs[4 * grp + j] = wr4[:, j * 64:(j + 1) * 64]

            # interleave A and B so the PE alternates G bursts and BD groups
            for hb in range(6):
                emit_g_block(hb)
            emit_bd(0)
            emit_bupdate(0)
            for hb in range(6, 9):
                emit_g_block(hb)
            emit_bd(1)
            emit_bupdate(1)
            emit_wrep(0)
            for hb in range(9, 12):
                emit_g_block(hb)
            emit_bd(2)
            emit_bupdate(2)
            emit_wrep(1)
            for hb in range(12, 16):
                emit_g_block(hb)
            emit_bd(3)
            emit_bupdate(3)
            emit_wrep(2)
            emit_wrep(3)

            # Z^2 per c, replicated to [64, 128] (overlaps phase D)
            wsum = small.tile([128, 8], f32, tag="wsum", name="wsum")
            nc.vector.tensor_reduce(
                wsum,
                bass.AP(tensor=wexpb.tensor, offset=wexpb.offset,
                        ap=[wexpb.ap[0], [1, 8], [8, 32]]),
                axis=mybir.AxisListType.X, op=mybir.AluOpType.add,
            )
            ps_z = ps_misc.tile([1, 8], f32, tag="m", name="zps")
            nc.tensor.matmul(ps_z, lhsT=onesm, rhs=wsum, start=True, stop=True)
            zsq = small.tile([1, 8], f32, tag="zsq", name="zsq")
            nc.scalar.activation(zsq, ps_z, mybir.ActivationFunctionType.Square)
            zrow = small.tile([1, 128], f32, tag="zrow", name="zrow")
            nc.scalar.activation(
                zrow.rearrange("p (o c) -> p o c", c=8),
                bass.AP(tensor=zsq.tensor, offset=zsq.offset,
                        ap=[zsq.ap[0], [0, 16], [1, 8]]),
                mybir.ActivationFunctionType.Copy,
            )
            ps_zq = ps_misc.tile([64, 128], f32, tag="m", name="zqps")
            nc.tensor.matmul(ps_zq, lhsT=onesrow, rhs=zrow, start=True, stop=True)
            zqsb = small.tile([64, 128], f32, tag="zqsb", name="zqsb")
            nc.scalar.activation(zqsb, ps_zq, mybir.ActivationFunctionType.Copy)

            # -- phase D: WW multiplies + col-paired n-matmuls --
            def emit_ww_n(hb):
                wr = wrs[hb]
                ww = wwpool.tile([128, BLK * 128], cdt, tag="ww", name="ww")
                in1 = bass.AP(tensor=wr.tensor, offset=wr.offset,
                              ap=[wr.ap[0], [8, 8], [0, 16], [1, 8]])
                eng = nc.gpsimd if hb in WW_GPS_SET else nc.vector
                eng.tensor_tensor(
                    ww.rearrange("p (l o c) -> p l o c", o=16, c=8),
                    wk_block(hb).rearrange("p l (o c) -> p l o c", c=8),
                    in1,
                    op=mybir.AluOpType.mult,
                )
                xts = xtb if it == 2 else xt8
                for lo in range(BLK):
                    t = hb * BLK + lo
                    half = t % 2
                    nc.tensor.matmul(ps_n[half * 64:(half + 1) * 64, :],
                                     lhsT=xts[:, t * 64:(t + 1) * 64],
                                     rhs=ww[:, lo * 128:(lo + 1) * 128],
                                     start=(t < 2), stop=(t >= NT - 2))

            for hb in range(NB):
                emit_ww_n(hb)

            if it < 2:
                squash_from(fold_n(ps_n), zqsb, True)
            else:
                out_sb = squash_from(fold_n(ps_n), zqsb, False)
                nc.sync.dma_start(out=out_d[:], in_=out_sb)

    nc.finalize()
    return nc


def _host_prep(x, W):
    """Build per-core input dicts."""
    import ml_dtypes
    ct = ml_dtypes.bfloat16
    f8 = ml_dtypes.float8_e4m3fn
    x = np.ascontiguousarray(x, dtype=np.float32)
    W = np.ascontiguousarray(W, dtype=np.float32)
    # xt[p=(q,i), t*64+b] = x[b, 16t+q, i]
    xt = x.reshape(B, NT, 16, I).transpose(2, 3, 1, 0).reshape(128, NT, 64)
    xt8 = np.ascontiguousarray(xt.reshape(128, NT * 64)).astype(f8)
    # xn8p[0:64, hb*512 + u*128 + (q*8+i)] = tile hb*8+u; rows 64:128 get
    # tiles hb*8+4+u (row-pair packing)
    xr = x.reshape(B, NB, 2, 4, 128)
    xn8p = np.concatenate([xr[:, :, 0], xr[:, :, 1]], axis=0)
    xn8p = np.ascontiguousarray(xn8p.reshape(128, NB * 512)).astype(f8)
    cstb, cstf = _consts_np()
    in_maps = []
    for k in range(N_CORES):
        Ws = W[:, k * CL:(k + 1) * CL]  # [R, 8, O, I]
        wkm = (Ws.reshape(NT, 16, CL, O, I).transpose(1, 4, 0, 3, 2)
               .reshape(128, NT, 128))
        im = {
            "xn8p": xn8p,
            "xt8": xt8,
            "xtb": np.ascontiguousarray(xt.reshape(128, NT * 64)).astype(ct),
            "cstb": cstb.astype(ct),
            "cstf": cstf,
        }
        for h in range(8):
            im[f"wk{h}"] = np.ascontiguousarray(
                wkm[:, 16 * h:16 * (h + 1), :].reshape(128, 2048)).astype(ct)
        in_maps.append(im)
    return in_maps


_CACHE = {}


def _get_nc():
    if "nc" not in _CACHE:
        _CACHE["nc"] = build_bass()
    return _CACHE["nc"]


def run(x, W, trace=False):
    nc = _get_nc()
    in_maps = _host_prep(x, W)
    res = run_bass_kernel_spmd(nc, in_maps, core_ids=list(range(N_CORES)),
                               trace=trace)
    outs = [np.asarray(res.results[k]["out"], dtype=np.float32)
            for k in range(N_CORES)]
    # out[b, (o, c)]: core k holds capsules [8k, 8k+8)
    v = np.concatenate(
        [o.reshape(B, O, CL).transpose(0, 2, 1) for o in outs], axis=1)
    return v[..., None], res


def kernel(x, W):
    v, _ = run(np.asarray(x), np.asarray(W))
    return v


# revision 7
# speedup vs baseline: 1.1187x; 1.1187x over previous
"""DigitCapsule dynamic-routing kernel for 8 TRN2 NeuronCores.

Strategy: the reference routing is fully independent per output capsule c
(softmax over routes, sums over routes, batch-mean are all per-c). So we
shard the C=64 capsules 8-ways: each core gets W[:, 8k:8k+8] and a
replicated x. Zero collectives; identical SPMD program per core with
per-core inputs.

Per core (B=64, R=2048, I=8, CL=8, O=16; K-dim = (r,i) = 16384 = 128
k-tiles of 128 = (16 routes q, 8 i)). s/v tensors live as
[b=64, (o,c)=128]; routing state lives banded as [(j,q)=128, (g,lo,c)].

  pass 0:  n0[b,(o,c)] = sum_t xt_t^T @ wk_t    (c_ij uniform), with
           col-paired matmuls (even k-tiles -> psum rows 0:64, odd ->
           64:128; fold with one cross-base add)
           v = n|n| / (R^2 + n^2)       == squash(n/R), exact algebra
  iter 1,2:
    A: G[(q,i),(lo,(o,c))] = xn^T @ V, row-paired: two concurrent K=64
       matmuls (fp8 x halves on partitions 0:64 / 64:128) into the two
       psum banks of each block; per block the P = G (.) Wk multiply
       runs on DVE (direct from psum) or ACT-drain + DVE/GPS
    B: BD-matmul bands psb[(j,q),(lo,o,c)] per grp (interleaved with A's
       G-matmuls so the PE never idles); ored = reduce_o; bstate +=
       ored/B; wexpb = exp(bstate); wrep matmuls merged 4-per-psum-tile
    D: WW = Wr (.) wrep (broadcast o) on DVE/GPS; n += xt_t^T @ WW_t
       col-paired
    Z[c] = sum_r wexp;  v = n|n| / (Z^2 + n^2)  == squash(n/Z), exact
  out[b,(o,c)] = v (f32)

V is duplicated into both partition halves of Vz via a small replication
matmul (I2) so the row-paired G matmuls can read per-half rhs.
"""

import os
import sys

for _p in ("/opt/trn_rl_repo", "/root/.axon_site/_ro/trn_rl_repo"):
    if os.path.isdir(_p) and _p not in sys.path:
        sys.path.insert(0, _p)

from contextlib import ExitStack

import numpy as np

import concourse.bass as bass
import concourse.bacc as bacc
from concourse import mybir
from concourse.bass_utils import run_bass_kernel_spmd
from concourse.tile import TileContext

B, R, C, O, I = 64, 2048, 64, 16, 8
N_CORES = 8
CL = C // N_CORES            # capsules per core = 8
F = CL * O                   # free (o,c) = 128
NT = R // 16                 # 128 k-tiles; tile t = routes [16t,16t+16), part p=(q,i)
NB = 16                      # number of 8-k-tile blocks
BLK = NT // NB               # 8 k-tiles per block

# P-production path per block: DVE-direct from psum / ACT drain + DVE /
# ACT drain + GPS
# GPS tensor ops starve DVE of SBUF bandwidth (measured: a concurrent GPS
# TT slows an identical DVE TT from 690ns to ~2500ns), so all elementwise
# work stays on DVE with ACT doing the PSUM drains.
DIRECT_SET = set()
P_GPS_SET = set()
WW_GPS_SET = set()


def _consts_np():
    """cstb [128,1152] bf16: BDF4 [0:512), BDT [512:1024), I2 dup [1024:1152).
    cstf [128,65] f32: masked-ones col 0; ones-row (partition 0) cols [1:65)."""
    cstb = np.zeros((128, 1216), dtype=np.float32)
    p = np.arange(128)
    # BDF4_j[p=(q,i), m] = 1 iff m == 32j + p//8  (i-reduce into band 32j+q)
    for j in range(4):
        cstb[p, 128 * j + 32 * j + p // 8] = 1.0
    # BDT_j = BDF4_j^T (band (j,q) -> rows (q,i))
    for j in range(4):
        cstb[:, 512 + 128 * j:512 + 128 * (j + 1)] = \
            cstb[:, 128 * j:128 * (j + 1)].T
    # I2[b, m] = 1 iff m % 64 == b (for b < 64): psum dup of V to both halves
    cstb[p[:64], 1024 + p[:64]] = 1.0
    cstb[p[:64], 1024 + 64 + p[:64]] = 1.0
    cstf = np.zeros((128, 65), dtype=np.float32)
    # Z-reduce mask: only band rows 32j+q (q<16) hold real data; the other
    # 64 partitions of wexpb are exp(0)=1 junk and must not enter Z.
    cstf[p[(p % 32) < 16], 0] = 1.0
    cstf[0, 1:65] = 1.0
    return cstb, cstf


def build_bass():
    f32 = mybir.dt.float32
    cdt = mybir.dt.bfloat16
    f8 = mybir.dt.float8e4

    nc = bacc.Bacc()
    # wk: 8 chunks of 2048 cols of W (bf16), k-tile t at chunk t//16
    wk_d = [nc.declare_dram_parameter(f"wk{h}", [128, 2048], cdt, isOutput=False)
            for h in range(8)]
    # xn8p: fp8 x, pair-packed: rows 0:64 = tiles hb*8+{0..3}, rows 64:128
    # = tiles hb*8+{4..7} (each block hb owns 512 cols)
    xn8p_d = nc.declare_dram_parameter("xn8p", [128, NB * 512], f8, isOutput=False)
    # xt8: fp8 x in (q,i)-partition layout, tile t at cols [64t, 64t+64)
    xt8_d = nc.declare_dram_parameter("xt8", [128, NT * 64], f8, isOutput=False)
    # xtb: bf16 x, same layout; loaded mid-kernel for the final n-pass
    xtb_d = nc.declare_dram_parameter("xtb", [128, NT * 64], cdt, isOutput=False)
    cstb_d = nc.declare_dram_parameter("cstb", [128, 1216], cdt, isOutput=False)
    cstf_d = nc.declare_dram_parameter("cstf", [128, 65], f32, isOutput=False)
    out_d = nc.declare_dram_parameter("out", [B, F], f32, isOutput=True)

    with TileContext(nc) as tc, ExitStack() as ctx:
        big = ctx.enter_context(tc.tile_pool(name="big", bufs=1))
        small = ctx.enter_context(tc.tile_pool(name="small", bufs=3))
        pgpool = ctx.enter_context(tc.tile_pool(name="pgpool", bufs=3))
        p16 = ctx.enter_context(tc.tile_pool(name="p16", bufs=NB + 1))
        wwpool = ctx.enter_context(tc.tile_pool(name="wwpool", bufs=4))
        ps_acc = ctx.enter_context(tc.tile_pool(name="ps_acc", bufs=1, space="PSUM"))
        ps_gb = ctx.enter_context(tc.tile_pool(name="ps_gb", bufs=3, space="PSUM"))
        ps_misc = ctx.enter_context(tc.tile_pool(name="ps_misc", bufs=1, space="PSUM"))

        # ---- load inputs: many small pieces round-robined across the
        # sync/gpsimd/scalar issue queues (per-stream DMA bw is low; the
        # aggregate needs ~20 concurrent streams). Priority order: consts,
        # xt8 + first wk chunks (pass0 critical path), xn8p, rest of wk,
        # then the bf16 xtb which is only needed in iter 2's n-pass.
        cstb = big.tile([128, 1216], cdt, tag="cstb", name="cstb")
        cstf = big.tile([128, 65], f32, tag="cstf", name="cstf")
        xt8 = big.tile([128, NT * 64], f8, tag="xt8", name="xt8")
        wk = [big.tile([128, 2048], cdt, tag=f"wk{h}", name=f"wk{h}")
              for h in range(8)]
        xn8p = big.tile([128, NB * 512], f8, tag="xn8p", name="xn8p")
        xtb = big.tile([128, NT * 64], cdt, tag="xtb", name="xtb")
        dma_q = [nc.sync, nc.gpsimd, nc.scalar]
        jobs = [(cstb, cstb_d[:]), (cstf, cstf_d[:])]
        for piece in range(4):
            c0 = piece * 2048
            jobs.append((xt8[:, c0:c0 + 2048], xt8_d[:, c0:c0 + 2048]))
        for h in range(4):
            for piece in range(2):
                c0 = piece * 1024
                jobs.append((wk[h][:, c0:c0 + 1024], wk_d[h][:, c0:c0 + 1024]))
        for piece in range(4):
            c0 = piece * 2048
            jobs.append((xn8p[:, c0:c0 + 2048], xn8p_d[:, c0:c0 + 2048]))
        for h in range(4, 8):
            for piece in range(2):
                c0 = piece * 1024
                jobs.append((wk[h][:, c0:c0 + 1024], wk_d[h][:, c0:c0 + 1024]))
        for idx, (dst, srcp) in enumerate(jobs):
            dma_q[idx % 3].dma_start(out=dst, in_=srcp)

        BDF4 = cstb[:, 0:512]
        BDT = cstb[:, 512:1024]
        I2 = cstb[0:64, 1024:1152]
        ZW = cstb[:, 1152:1216]      # zero lhsT for HAM-warm dummy matmuls
        onesm = cstf[:, 0:1]
        onesrow = cstf[0:1, 1:65]

        def wk_tile(t):
            h, lo = t // 16, t % 16
            return wk[h][:, lo * 128:(lo + 1) * 128]

        def wk_block(hb):
            # [128, 8, 128] view of block hb's 8 k-tiles of W
            wkh = wk[hb // 2].rearrange("p (u f) -> p u f", f=128)
            return wkh[:, (hb % 2) * BLK:(hb % 2) * BLK + BLK, :]

        # V: [128,128] bf16; squash writes rows 0:64, dup-matmul fills 64:128
        Vz = big.tile([128, 128], cdt, tag="Vz", name="Vz")

        # v = n*|n| / (zsq + n^2); nf is [64,128] f32 in SBUF
        def squash_from(nf, zsq_sb, mk_V):
            absn = small.tile([64, 128], f32, tag="absn", name="absn")
            nc.scalar.activation(absn, nf, mybir.ActivationFunctionType.Abs)
            nsq = small.tile([64, 128], f32, tag="nsq", name="nsq")
            nc.scalar.activation(nsq, nf, mybir.ActivationFunctionType.Square)
            den = small.tile([64, 128], f32, tag="den", name="den")
            if zsq_sb is None:
                nc.vector.tensor_scalar_add(den, nsq, float(R) * float(R))
            else:
                nc.vector.tensor_add(den, nsq, zsq_sb)
            rden = small.tile([64, 128], f32, tag="rden", name="rden")
            nc.vector.reciprocal_approx_fast(rden, den)
            num = small.tile([64, 128], f32, tag="num", name="num")
            nc.vector.tensor_mul(num, nf, absn)
            if not mk_V:
                out_sb = small.tile([64, 128], f32, tag="outsb", name="outsb")
                nc.vector.tensor_mul(out_sb, num, rden)
                return out_sb
            nc.vector.tensor_mul(Vz[0:64, :], num, rden)
            # duplicate V into rows 64:128 via replication matmul
            ps_dup = ps_misc.tile([128, 128], f32, tag="m", name="dup")
            nc.tensor.matmul(ps_dup, lhsT=I2, rhs=Vz[0:64, :],
                             start=True, stop=True)
            nc.scalar.activation(Vz[64:128, :], ps_dup[64:128, :],
                                 mybir.ActivationFunctionType.Copy)
            return None

        # fold the two col-pair accumulator halves and return n as f32 SBUF
        def fold_n(ps_n):
            nhi = small.tile([64, 128], f32, tag="nhi", name="nhi")
            nc.scalar.activation(nhi, ps_n[64:128, :],
                                 mybir.ActivationFunctionType.Copy)
            nf = small.tile([64, 128], f32, tag="nf", name="nf")
            nc.vector.tensor_add(nf, ps_n[0:64, :], nhi)
            return nf

        # ---- pass 0: n0 = sum_t xt8_t^T @ wk_t (col-paired) ; V = squash ----
        ps_s = ps_acc.tile([128, 128], f32, tag="acc", name="acc")
        for t in range(NT):
            half = t % 2
            nc.tensor.matmul(ps_s[half * 64:(half + 1) * 64, :],
                             lhsT=xt8[:, t * 64:(t + 1) * 64],
                             rhs=wk_tile(t),
                             start=(t < 2), stop=(t >= NT - 2))
        squash_from(fold_n(ps_s), None, True)
        # HAM-warm dummies across the squash gap (zero lhsT accumulates
        # nothing; rhs choices pace them behind pass0 / squash results)
        for wd in range(4):
            nc.tensor.matmul(ps_s[0:64, :], lhsT=ZW,
                             rhs=wk[7][:, wd * 128:(wd + 1) * 128],
                             start=False, stop=False, skip_group_check=True)
        for wd in range(2):
            nc.tensor.matmul(ps_s[0:64, :], lhsT=ZW[0:64, :], rhs=Vz[0:64, :],
                             start=False, stop=False, skip_group_check=True)
        # xtb (bf16 x for iter2's n-pass) loads only now: the issues sit on
        # the scalar queue behind pass0-dependent work, so the transfer does
        # not compete with the critical input phase.
        for piece in range(4):
            c0 = piece * 2048
            nc.scalar.dma_start(out=xtb[:, c0:c0 + 2048],
                                in_=xtb_d[:, c0:c0 + 2048])

        bstate = small.tile([128, 256], f32, tag="bstate", name="bstate", bufs=1)
        nc.vector.memset(bstate, 0.0)
        wexpb = small.tile([128, 256], cdt, tag="wexpb", name="wexpb", bufs=1)

        for it in (1, 2):
            ps_n = ps_acc.tile([128, 128], f32, tag="acc", name="acc")
            Ps = [None] * NB
            psbs = [None] * 4
            wrs = [None] * NB

            # -- phase A pieces: G row-pairs + P production for one block --
            def emit_g_block(hb):
                psg = ps_gb.tile([128, BLK * 128], f32, tag="gb", name="gb")
                for u in range(4):
                    cs = slice(hb * 512 + u * 128, hb * 512 + (u + 1) * 128)
                    nc.tensor.matmul(psg[:, u * 128:(u + 1) * 128],
                                     lhsT=xn8p[0:64, cs], rhs=Vz[0:64, :],
                                     start=True, stop=True)
                    nc.tensor.matmul(psg[:, 512 + u * 128:512 + (u + 1) * 128],
                                     lhsT=xn8p[64:128, cs], rhs=Vz[64:128, :],
                                     start=True, stop=True)
                P = p16.tile([128, BLK * 128], cdt, tag="P", name="P")
                if hb in DIRECT_SET:
                    nc.vector.tensor_tensor(
                        P.rearrange("p (u f) -> p u f", f=128),
                        psg.rearrange("p (u f) -> p u f", f=128),
                        wk_block(hb),
                        op=mybir.AluOpType.mult,
                    )
                else:
                    Pg = pgpool.tile([128, BLK * 128], cdt, tag="Pg", name="Pg")
                    nc.scalar.activation(Pg, psg,
                                         mybir.ActivationFunctionType.Copy)
                    eng = nc.gpsimd if hb in P_GPS_SET else nc.vector
                    eng.tensor_tensor(
                        P.rearrange("p (u f) -> p u f", f=128),
                        Pg.rearrange("p (u f) -> p u f", f=128),
                        wk_block(hb),
                        op=mybir.AluOpType.mult,
                    )
                Ps[hb] = P
                if hb % 2 == 1:
                    nc.tensor.matmul(ps_n[0:64, :], lhsT=ZW,
                                     rhs=Ps[hb - 1][:, 0:128],
                                     start=False, stop=False,
                                     skip_group_check=True)

            # -- phase B pieces --
            def emit_bd(grp):
                psb = ps_gb.tile([128, BLK * 128], f32, tag="gb", name="gb")
                for j in range(4):
                    for half in range(2):
                        nc.tensor.matmul(
                            psb[:, half * 512:(half + 1) * 512],
                            lhsT=BDF4[:, 128 * j:128 * (j + 1)],
                            rhs=Ps[4 * grp + j][:, half * 512:(half + 1) * 512],
                            start=(j == 0), stop=(j == 3),
                        )
                psbs[grp] = psb

            def emit_bupdate(grp):
                ored = small.tile([128, 64], f32, tag="ored", name="ored",
                                  bufs=2)
                psb = psbs[grp]
                nc.vector.tensor_reduce(
                    ored.rearrange("p (l c) -> p l c", c=8),
                    bass.AP(tensor=psb.tensor, offset=psb.offset,
                            ap=[psb.ap[0], [128, 8], [1, 8], [8, 16]]),
                    axis=mybir.AxisListType.X,
                    op=mybir.AluOpType.add,
                )
                cs = slice(grp * 64, (grp + 1) * 64)
                nc.vector.scalar_tensor_tensor(bstate[:, cs], ored, 1.0 / B,
                                               bstate[:, cs],
                                               op0=mybir.AluOpType.mult,
                                               op1=mybir.AluOpType.add)
                nc.scalar.activation(wexpb[:, cs], bstate[:, cs],
                                     mybir.ActivationFunctionType.Exp)

            def emit_wrep(grp):
                cs = slice(grp * 64, (grp + 1) * 64)
                ps_wr = ps_misc.tile([128, 256], f32, tag="m", name="wrps")
                for j in range(4):
                    nc.tensor.matmul(ps_wr[:, j * 64:(j + 1) * 64],
                                     lhsT=BDT[:, 128 * j:128 * (j + 1)],
                                     rhs=wexpb[:, cs], start=True, stop=True)
                wr4 = small.tile([128, 256], cdt, tag="wr", name="wr", bufs=2)
                nc.scalar.activation(wr4, ps_wr,
                                     mybir.ActivationFunctionType.Copy)
                for j in range(4):
                    wrs[4 * grp + j] = wr4[:, j * 64:(j + 1) * 64]

            # interleave A and B so the PE alternates G bursts and BD groups
            for hb in range(6):
                emit_g_block(hb)
            emit_bd(0)
            emit_bupdate(0)
            for hb in range(6, 9):
                emit_g_block(hb)
            emit_bd(1)
            emit_bupdate(1)
            emit_wrep(0)
            for hb in range(9, 12):
                emit_g_block(hb)
            emit_bd(2)
            emit_bupdate(2)
            emit_wrep(1)
            for hb in range(12, 16):
                emit_g_block(hb)
            emit_bd(3)
            emit_bupdate(3)
            emit_wrep(2)
            emit_wrep(3)

            # Z^2 per c, replicated to [64, 128] (overlaps phase D)
            wsum = small.tile([128, 8], f32, tag="wsum", name="wsum")
            nc.vector.tensor_reduce(
                wsum,
                bass.AP(tensor=wexpb.tensor, offset=wexpb.offset,
                        ap=[wexpb.ap[0], [1, 8], [8, 32]]),
                axis=mybir.AxisListType.X, op=mybir.AluOpType.add,
            )
            ps_z = ps_misc.tile([1, 8], f32, tag="m", name="zps")
            nc.tensor.matmul(ps_z, lhsT=onesm, rhs=wsum, start=True, stop=True)
            zsq = small.tile([1, 8], f32, tag="zsq", name="zsq")
            nc.scalar.activation(zsq, ps_z, mybir.ActivationFunctionType.Square)
            zrow = small.tile([1, 128], f32, tag="zrow", name="zrow")
            nc.scalar.activation(
                zrow.rearrange("p (o c) -> p o c", c=8),
                bass.AP(tensor=zsq.tensor, offset=zsq.offset,
                        ap=[zsq.ap[0], [0, 16], [1, 8]]),
                mybir.ActivationFunctionType.Copy,
            )
            ps_zq = ps_misc.tile([64, 128], f32, tag="m", name="zqps")
            nc.tensor.matmul(ps_zq, lhsT=onesrow, rhs=zrow, start=True, stop=True)
            zqsb = small.tile([64, 128], f32, tag="zqsb", name="zqsb")
            nc.scalar.activation(zqsb, ps_zq, mybir.ActivationFunctionType.Copy)

            # -- phase D: WW multiplies + col-paired n-matmuls --
            def emit_ww_n(hb):
                wr = wrs[hb]
                ww = wwpool.tile([128, BLK * 128], cdt, tag="ww", name="ww")
                in1 = bass.AP(tensor=wr.tensor, offset=wr.offset,
                              ap=[wr.ap[0], [8, 8], [0, 16], [1, 8]])
                eng = nc.gpsimd if hb in WW_GPS_SET else nc.vector
                eng.tensor_tensor(
                    ww.rearrange("p (l o c) -> p l o c", o=16, c=8),
                    wk_block(hb).rearrange("p l (o c) -> p l o c", c=8),
                    in1,
                    op=mybir.AluOpType.mult,
                )
                xts = xtb if it == 2 else xt8
                for lo in range(BLK):
                    t = hb * BLK + lo
                    half = t % 2
                    nc.tensor.matmul(ps_n[half * 64:(half + 1) * 64, :],
                                     lhsT=xts[:, t * 64:(t + 1) * 64],
                                     rhs=ww[:, lo * 128:(lo + 1) * 128],
                                     start=(t < 2), stop=(t >= NT - 2))

            for hb in range(NB):
                emit_ww_n(hb)

            if it < 2:
                squash_from(fold_n(ps_n), zqsb, True)
                for wd in range(2):
                    nc.tensor.matmul(ps_n[0:64, :], lhsT=ZW[0:64, :],
                                     rhs=Vz[0:64, :], start=False,
                                     stop=False, skip_group_check=True)
            else:
                out_sb = squash_from(fold_n(ps_n), zqsb, False)
                nc.sync.dma_start(out=out_d[:], in_=out_sb)

    nc.finalize()
    return nc


def _host_prep(x, W):
    """Build per-core input dicts."""
    import ml_dtypes
    ct = ml_dtypes.bfloat16
    f8 = ml_dtypes.float8_e4m3fn
    x = np.ascontiguousarray(x, dtype=np.float32)
    W = np.ascontiguousarray(W, dtype=np.float32)
    # xt[p=(q,i), t*64+b] = x[b, 16t+q, i]
    xt = x.reshape(B, NT, 16, I).transpose(2, 3, 1, 0).reshape(128, NT, 64)
    xt8 = np.ascontiguousarray(xt.reshape(128, NT * 64)).astype(f8)
    # xn8p[0:64, hb*512 + u*128 + (q*8+i)] = tile hb*8+u; rows 64:128 get
    # tiles hb*8+4+u (row-pair packing)
    xr = x.reshape(B, NB, 2, 4, 128)
    xn8p = np.concatenate([xr[:, :, 0], xr[:, :, 1]], axis=0)
    xn8p = np.ascontiguousarray(xn8p.reshape(128, NB * 512)).astype(f8)
    cstb, cstf = _consts_np()
    in_maps = []
    for k in range(N_CORES):
        Ws = W[:, k * CL:(k + 1) * CL]  # [R, 8, O, I]
        wkm = (Ws.reshape(NT, 16, CL, O, I).transpose(1, 4, 0, 3, 2)
               .reshape(128, NT, 128))
        im = {
            "xn8p": xn8p,
            "xt8": xt8,
            "xtb": np.ascontiguousarray(xt.reshape(128, NT * 64)).astype(ct),
            "cstb": cstb.astype(ct),
            "cstf": cstf,
        }
        for h in range(8):
            im[f"wk{h}"] = np.ascontiguousarray(
                wkm[:, 16 * h:16 * (h + 1), :].reshape(128, 2048)).astype(ct)
        in_maps.append(im)
    return in_maps


_CACHE = {}


def _get_nc():
    if "nc" not in _CACHE:
        _CACHE["nc"] = build_bass()
    return _CACHE["nc"]


def run(x, W, trace=False):
    nc = _get_nc()
    in_maps = _host_prep(x, W)
    res = run_bass_kernel_spmd(nc, in_maps, core_ids=list(range(N_CORES)),
                               trace=trace)
    outs = [np.asarray(res.results[k]["out"], dtype=np.float32)
            for k in range(N_CORES)]
    # out[b, (o, c)]: core k holds capsules [8k, 8k+8)
    v = np.concatenate(
        [o.reshape(B, O, CL).transpose(0, 2, 1) for o in outs], axis=1)
    return v[..., None], res


def kernel(x, W):
    v, _ = run(np.asarray(x), np.asarray(W))
    return v


# revision 8
# speedup vs baseline: 1.1251x; 1.0057x over previous
"""DigitCapsule dynamic-routing kernel for 8 TRN2 NeuronCores.

Strategy: the reference routing is fully independent per output capsule c
(softmax over routes, sums over routes, batch-mean are all per-c). So we
shard the C=64 capsules 8-ways: each core gets W[:, 8k:8k+8] and a
replicated x. Zero collectives; identical SPMD program per core with
per-core inputs.

Per core (B=64, R=2048, I=8, CL=8, O=16; K-dim = (r,i) = 16384 = 128
k-tiles of 128 = (16 routes q, 8 i)). s/v tensors live as
[b=64, (o,c)=128]; routing state lives banded as [(j,q)=128, (g,lo,c)].

  pass 0:  n0[b,(o,c)] = sum_t xt_t^T @ wk_t    (c_ij uniform), with
           col-paired matmuls (even k-tiles -> psum rows 0:64, odd ->
           64:128; fold with one cross-base add)
           v = n|n| / (R^2 + n^2)       == squash(n/R), exact algebra
  iter 1,2:
    A: G[(q,i),(lo,(o,c))] = xn^T @ V, row-paired: two concurrent K=64
       matmuls (fp8 x halves on partitions 0:64 / 64:128) into the two
       psum banks of each block; per block the P = G (.) Wk multiply
       runs on DVE (direct from psum) or ACT-drain + DVE/GPS
    B: BD-matmul bands psb[(j,q),(lo,o,c)] per grp (interleaved with A's
       G-matmuls so the PE never idles); ored = reduce_o; bstate +=
       ored/B; wexpb = exp(bstate); wrep matmuls merged 4-per-psum-tile
    D: WW = Wr (.) wrep (broadcast o) on DVE/GPS; n += xt_t^T @ WW_t
       col-paired
    Z[c] = sum_r wexp;  v = n|n| / (Z^2 + n^2)  == squash(n/Z), exact
  out[b,(o,c)] = v (f32)

V is duplicated into both partition halves of Vz via a small replication
matmul (I2) so the row-paired G matmuls can read per-half rhs.
"""

import os
import sys

for _p in ("/opt/trn_rl_repo", "/root/.axon_site/_ro/trn_rl_repo"):
    if os.path.isdir(_p) and _p not in sys.path:
        sys.path.insert(0, _p)

from contextlib import ExitStack

import numpy as np

import concourse.bass as bass
import concourse.bacc as bacc
from concourse import mybir
from concourse.bass_utils import run_bass_kernel_spmd
from concourse.tile import TileContext

B, R, C, O, I = 64, 2048, 64, 16, 8
N_CORES = 8
CL = C // N_CORES            # capsules per core = 8
F = CL * O                   # free (o,c) = 128
NT = R // 16                 # 128 k-tiles; tile t = routes [16t,16t+16), part p=(q,i)
NB = 16                      # number of 8-k-tile blocks
BLK = NT // NB               # 8 k-tiles per block

# P-production path per block: DVE-direct from psum / ACT drain + DVE /
# ACT drain + GPS
# GPS tensor ops starve DVE of SBUF bandwidth (measured: a concurrent GPS
# TT slows an identical DVE TT from 690ns to ~2500ns), so all elementwise
# work stays on DVE with ACT doing the PSUM drains.
DIRECT_SET = {0, 5, 9, 13}
P_GPS_SET = set()
WW_GPS_SET = set()


def _consts_np():
    """cstb [128,1152] bf16: BDF4 [0:512), BDT [512:1024), I2 dup [1024:1152).
    cstf [128,65] f32: masked-ones col 0; ones-row (partition 0) cols [1:65)."""
    cstb = np.zeros((128, 1216), dtype=np.float32)
    p = np.arange(128)
    # BDF4_j[p=(q,i), m] = 1 iff m == 32j + p//8  (i-reduce into band 32j+q)
    for j in range(4):
        cstb[p, 128 * j + 32 * j + p // 8] = 1.0
    # BDT_j = BDF4_j^T (band (j,q) -> rows (q,i))
    for j in range(4):
        cstb[:, 512 + 128 * j:512 + 128 * (j + 1)] = \
            cstb[:, 128 * j:128 * (j + 1)].T
    # I2[b, m] = 1 iff m % 64 == b (for b < 64): psum dup of V to both halves
    cstb[p[:64], 1024 + p[:64]] = 1.0
    cstb[p[:64], 1024 + 64 + p[:64]] = 1.0
    cstf = np.zeros((128, 65), dtype=np.float32)
    # Z-reduce mask: only band rows 32j+q (q<16) hold real data; the other
    # 64 partitions of wexpb are exp(0)=1 junk and must not enter Z.
    cstf[p[(p % 32) < 16], 0] = 1.0
    cstf[0, 1:65] = 1.0
    return cstb, cstf


def build_bass():
    f32 = mybir.dt.float32
    cdt = mybir.dt.bfloat16
    f8 = mybir.dt.float8e4

    nc = bacc.Bacc()
    # wk: 8 chunks of 2048 cols of W (bf16), k-tile t at chunk t//16
    wk_d = [nc.declare_dram_parameter(f"wk{h}", [128, 2048], cdt, isOutput=False)
            for h in range(8)]
    # xn8p: fp8 x, pair-packed: rows 0:64 = tiles hb*8+{0..3}, rows 64:128
    # = tiles hb*8+{4..7} (each block hb owns 512 cols)
    xn8p_d = nc.declare_dram_parameter("xn8p", [128, NB * 512], f8, isOutput=False)
    # xt8: fp8 x in (q,i)-partition layout, tile t at cols [64t, 64t+64)
    xt8_d = nc.declare_dram_parameter("xt8", [128, NT * 64], f8, isOutput=False)
    # xtb: bf16 x, same layout; loaded mid-kernel for the final n-pass
    xtb_d = nc.declare_dram_parameter("xtb", [128, NT * 64], cdt, isOutput=False)
    cstb_d = nc.declare_dram_parameter("cstb", [128, 1216], cdt, isOutput=False)
    cstf_d = nc.declare_dram_parameter("cstf", [128, 65], f32, isOutput=False)
    out_d = nc.declare_dram_parameter("out", [B, F], f32, isOutput=True)

    with TileContext(nc) as tc, ExitStack() as ctx:
        big = ctx.enter_context(tc.tile_pool(name="big", bufs=1))
        small = ctx.enter_context(tc.tile_pool(name="small", bufs=3))
        pgpool = ctx.enter_context(tc.tile_pool(name="pgpool", bufs=3))
        p16 = ctx.enter_context(tc.tile_pool(name="p16", bufs=NB + 1))
        wwpool = ctx.enter_context(tc.tile_pool(name="wwpool", bufs=4))
        ps_acc = ctx.enter_context(tc.tile_pool(name="ps_acc", bufs=1, space="PSUM"))
        ps_gb = ctx.enter_context(tc.tile_pool(name="ps_gb", bufs=3, space="PSUM"))
        ps_misc = ctx.enter_context(tc.tile_pool(name="ps_misc", bufs=1, space="PSUM"))

        # ---- load inputs: many small pieces round-robined across the
        # sync/gpsimd/scalar issue queues (per-stream DMA bw is low; the
        # aggregate needs ~20 concurrent streams). Priority order: consts,
        # xt8 + first wk chunks (pass0 critical path), xn8p, rest of wk,
        # then the bf16 xtb which is only needed in iter 2's n-pass.
        cstb = big.tile([128, 1216], cdt, tag="cstb", name="cstb")
        cstf = big.tile([128, 65], f32, tag="cstf", name="cstf")
        xt8 = big.tile([128, NT * 64], f8, tag="xt8", name="xt8")
        wk = [big.tile([128, 2048], cdt, tag=f"wk{h}", name=f"wk{h}")
              for h in range(8)]
        xn8p = big.tile([128, NB * 512], f8, tag="xn8p", name="xn8p")
        xtb = big.tile([128, NT * 64], cdt, tag="xtb", name="xtb")
        dma_q = [nc.sync, nc.gpsimd, nc.scalar]
        jobs = [(cstb, cstb_d[:]), (cstf, cstf_d[:])]
        for piece in range(4):
            c0 = piece * 2048
            jobs.append((xt8[:, c0:c0 + 2048], xt8_d[:, c0:c0 + 2048]))
        for h in range(8):
            for piece in range(4):
                c0 = piece * 512
                jobs.append((wk[h][:, c0:c0 + 512], wk_d[h][:, c0:c0 + 512]))
        for piece in range(4):
            c0 = piece * 2048
            jobs.append((xn8p[:, c0:c0 + 2048], xn8p_d[:, c0:c0 + 2048]))
        for idx, (dst, srcp) in enumerate(jobs):
            dma_q[idx % 3].dma_start(out=dst, in_=srcp)

        BDF4 = cstb[:, 0:512]
        BDT = cstb[:, 512:1024]
        I2 = cstb[0:64, 1024:1152]
        ZW = cstb[:, 1152:1216]      # zero lhsT for HAM-warm dummy matmuls
        onesm = cstf[:, 0:1]
        onesrow = cstf[0:1, 1:65]

        def wk_tile(t):
            h, lo = t // 16, t % 16
            return wk[h][:, lo * 128:(lo + 1) * 128]

        def wk_block(hb):
            # [128, 8, 128] view of block hb's 8 k-tiles of W
            wkh = wk[hb // 2].rearrange("p (u f) -> p u f", f=128)
            return wkh[:, (hb % 2) * BLK:(hb % 2) * BLK + BLK, :]

        # V: [128,128] bf16; squash writes rows 0:64, dup-matmul fills 64:128
        Vz = big.tile([128, 128], cdt, tag="Vz", name="Vz")

        # v = n*|n| / (zsq + n^2); nf is [64,128] f32 in SBUF
        def squash_from(nf, zsq_sb, mk_V):
            absn = small.tile([64, 128], f32, tag="absn", name="absn")
            nc.scalar.activation(absn, nf, mybir.ActivationFunctionType.Abs)
            nsq = small.tile([64, 128], f32, tag="nsq", name="nsq")
            nc.scalar.activation(nsq, nf, mybir.ActivationFunctionType.Square)
            den = small.tile([64, 128], f32, tag="den", name="den")
            if zsq_sb is None:
                nc.vector.tensor_scalar_add(den, nsq, float(R) * float(R))
            else:
                nc.vector.tensor_add(den, nsq, zsq_sb)
            rden = small.tile([64, 128], f32, tag="rden", name="rden")
            nc.vector.reciprocal_approx_fast(rden, den)
            num = small.tile([64, 128], f32, tag="num", name="num")
            nc.vector.tensor_mul(num, nf, absn)
            if not mk_V:
                out_sb = small.tile([64, 128], f32, tag="outsb", name="outsb")
                nc.vector.tensor_mul(out_sb, num, rden)
                return out_sb
            nc.vector.tensor_mul(Vz[0:64, :], num, rden)
            # duplicate V into rows 64:128 via replication matmul
            ps_dup = ps_misc.tile([128, 128], f32, tag="m", name="dup")
            nc.tensor.matmul(ps_dup, lhsT=I2, rhs=Vz[0:64, :],
                             start=True, stop=True)
            nc.scalar.activation(Vz[64:128, :], ps_dup[64:128, :],
                                 mybir.ActivationFunctionType.Copy)
            return None

        # fold the two col-pair accumulator halves and return n as f32 SBUF
        def fold_n(ps_n):
            nhi = small.tile([64, 128], f32, tag="nhi", name="nhi")
            nc.scalar.activation(nhi, ps_n[64:128, :],
                                 mybir.ActivationFunctionType.Copy)
            nf = small.tile([64, 128], f32, tag="nf", name="nf")
            nc.vector.tensor_add(nf, ps_n[0:64, :], nhi)
            return nf

        # ---- pass 0: n0 = sum_t xt8_t^T @ wk_t (col-paired) ; V = squash ----
        ps_s = ps_acc.tile([128, 128], f32, tag="acc", name="acc")
        for t in range(NT):
            half = t % 2
            nc.tensor.matmul(ps_s[half * 64:(half + 1) * 64, :],
                             lhsT=xt8[:, t * 64:(t + 1) * 64],
                             rhs=wk_tile(t),
                             start=(t < 2), stop=(t >= NT - 2))
        squash_from(fold_n(ps_s), None, True)
        # HAM-warm dummies across the squash gap (zero lhsT accumulates
        # nothing; rhs choices pace them behind pass0 / squash results)
        for wd in range(4):
            nc.tensor.matmul(ps_s[0:64, :], lhsT=ZW,
                             rhs=wk[7][:, wd * 128:(wd + 1) * 128],
                             start=False, stop=False, skip_group_check=True)
        for wd in range(2):
            nc.tensor.matmul(ps_s[0:64, :], lhsT=ZW[0:64, :], rhs=Vz[0:64, :],
                             start=False, stop=False, skip_group_check=True)
        # xtb (bf16 x for iter2's n-pass) loads only now: the issues sit on
        # the scalar queue behind pass0-dependent work, so the transfer does
        # not compete with the critical input phase.
        for piece in range(4):
            c0 = piece * 2048
            nc.scalar.dma_start(out=xtb[:, c0:c0 + 2048],
                                in_=xtb_d[:, c0:c0 + 2048])

        bstate = small.tile([128, 256], f32, tag="bstate", name="bstate", bufs=1)
        nc.vector.memset(bstate, 0.0)
        wexpb = small.tile([128, 256], cdt, tag="wexpb", name="wexpb", bufs=1)

        for it in (1, 2):
            ps_n = ps_acc.tile([128, 128], f32, tag="acc", name="acc")
            Ps = [None] * NB
            psbs = [None] * 4
            wrs = [None] * NB

            # -- phase A pieces: G row-pairs + P production for one block --
            def emit_g_block(hb):
                psg = ps_gb.tile([128, BLK * 128], f32, tag="gb", name="gb")
                for u in range(4):
                    cs = slice(hb * 512 + u * 128, hb * 512 + (u + 1) * 128)
                    nc.tensor.matmul(psg[:, u * 128:(u + 1) * 128],
                                     lhsT=xn8p[0:64, cs], rhs=Vz[0:64, :],
                                     start=True, stop=True)
                    nc.tensor.matmul(psg[:, 512 + u * 128:512 + (u + 1) * 128],
                                     lhsT=xn8p[64:128, cs], rhs=Vz[64:128, :],
                                     start=True, stop=True)
                P = p16.tile([128, BLK * 128], cdt, tag="P", name="P")
                if hb in DIRECT_SET:
                    nc.vector.tensor_tensor(
                        P.rearrange("p (u f) -> p u f", f=128),
                        psg.rearrange("p (u f) -> p u f", f=128),
                        wk_block(hb),
                        op=mybir.AluOpType.mult,
                    )
                else:
                    Pg = pgpool.tile([128, BLK * 128], cdt, tag="Pg", name="Pg")
                    nc.scalar.activation(Pg, psg,
                                         mybir.ActivationFunctionType.Copy)
                    eng = nc.gpsimd if hb in P_GPS_SET else nc.vector
                    eng.tensor_tensor(
                        P.rearrange("p (u f) -> p u f", f=128),
                        Pg.rearrange("p (u f) -> p u f", f=128),
                        wk_block(hb),
                        op=mybir.AluOpType.mult,
                    )
                Ps[hb] = P
                if hb % 2 == 1:
                    nc.tensor.matmul(ps_n[0:64, :], lhsT=ZW,
                                     rhs=Ps[hb - 1][:, 0:128],
                                     start=False, stop=False,
                                     skip_group_check=True)

            # -- phase B pieces --
            def emit_bd(grp):
                psb = ps_gb.tile([128, BLK * 128], f32, tag="gb", name="gb")
                for j in range(4):
                    for half in range(2):
                        nc.tensor.matmul(
                            psb[:, half * 512:(half + 1) * 512],
                            lhsT=BDF4[:, 128 * j:128 * (j + 1)],
                            rhs=Ps[4 * grp + j][:, half * 512:(half + 1) * 512],
                            start=(j == 0), stop=(j == 3),
                        )
                psbs[grp] = psb

            def emit_bupdate(grp):
                ored = small.tile([128, 64], f32, tag="ored", name="ored",
                                  bufs=2)
                psb = psbs[grp]
                nc.vector.tensor_reduce(
                    ored.rearrange("p (l c) -> p l c", c=8),
                    bass.AP(tensor=psb.tensor, offset=psb.offset,
                            ap=[psb.ap[0], [128, 8], [1, 8], [8, 16]]),
                    axis=mybir.AxisListType.X,
                    op=mybir.AluOpType.add,
                )
                cs = slice(grp * 64, (grp + 1) * 64)
                nc.vector.scalar_tensor_tensor(bstate[:, cs], ored, 1.0 / B,
                                               bstate[:, cs],
                                               op0=mybir.AluOpType.mult,
                                               op1=mybir.AluOpType.add)
                nc.scalar.activation(wexpb[:, cs], bstate[:, cs],
                                     mybir.ActivationFunctionType.Exp)

            def emit_wrep(grp):
                cs = slice(grp * 64, (grp + 1) * 64)
                ps_wr = ps_misc.tile([128, 256], f32, tag="m", name="wrps")
                for j in range(4):
                    nc.tensor.matmul(ps_wr[:, j * 64:(j + 1) * 64],
                                     lhsT=BDT[:, 128 * j:128 * (j + 1)],
                                     rhs=wexpb[:, cs], start=True, stop=True)
                wr4 = small.tile([128, 256], cdt, tag="wr", name="wr", bufs=2)
                nc.scalar.activation(wr4, ps_wr,
                                     mybir.ActivationFunctionType.Copy)
                for j in range(4):
                    wrs[4 * grp + j] = wr4[:, j * 64:(j + 1) * 64]

            # interleave A and B so the PE alternates G bursts and BD groups
            for hb in range(6):
                emit_g_block(hb)
            emit_bd(0)
            emit_bupdate(0)
            for hb in range(6, 9):
                emit_g_block(hb)
            emit_bd(1)
            emit_bupdate(1)
            emit_wrep(0)
            for hb in range(9, 12):
                emit_g_block(hb)
            emit_bd(2)
            emit_bupdate(2)
            emit_wrep(1)
            for hb in range(12, 16):
                emit_g_block(hb)
            emit_bd(3)
            emit_bupdate(3)
            emit_wrep(2)
            emit_wrep(3)

            # -- phase D: WW multiplies + col-paired n-matmuls --
            def emit_ww_n(hb):
                wr = wrs[hb]
                ww = wwpool.tile([128, BLK * 128], cdt, tag="ww", name="ww")
                in1 = bass.AP(tensor=wr.tensor, offset=wr.offset,
                              ap=[wr.ap[0], [8, 8], [0, 16], [1, 8]])
                eng = nc.gpsimd if hb in WW_GPS_SET else nc.vector
                eng.tensor_tensor(
                    ww.rearrange("p (l o c) -> p l o c", o=16, c=8),
                    wk_block(hb).rearrange("p l (o c) -> p l o c", c=8),
                    in1,
                    op=mybir.AluOpType.mult,
                )
                if hb % 2 == 0:
                    nc.tensor.matmul(ps_n[0:64, :], lhsT=ZW,
                                     rhs=ww[:, 0:128], start=False,
                                     stop=False, skip_group_check=True)
                xts = xtb if it == 2 else xt8
                for lo in range(BLK):
                    t = hb * BLK + lo
                    half = t % 2
                    nc.tensor.matmul(ps_n[half * 64:(half + 1) * 64, :],
                                     lhsT=xts[:, t * 64:(t + 1) * 64],
                                     rhs=ww[:, lo * 128:(lo + 1) * 128],
                                     start=(t < 2), stop=(t >= NT - 2))

            zqsb = None
            for hb in range(NB):
                emit_ww_n(hb)
                if hb == 1:
                    # Z^2 per c, replicated to [64, 128] (overlaps phase D;
                    # emitted after two WW blocks so it doesn't head-block
                    # the vector queue)
                    wsum = small.tile([128, 8], f32, tag="wsum", name="wsum")
                    nc.vector.tensor_reduce(
                        wsum,
                        bass.AP(tensor=wexpb.tensor, offset=wexpb.offset,
                                ap=[wexpb.ap[0], [1, 8], [8, 32]]),
                        axis=mybir.AxisListType.X, op=mybir.AluOpType.add,
                    )
                    ps_z = ps_misc.tile([1, 8], f32, tag="m", name="zps")
                    nc.tensor.matmul(ps_z, lhsT=onesm, rhs=wsum,
                                     start=True, stop=True)
                    zsq = small.tile([1, 8], f32, tag="zsq", name="zsq")
                    nc.scalar.activation(zsq, ps_z,
                                         mybir.ActivationFunctionType.Square)
                    zrow = small.tile([1, 128], f32, tag="zrow", name="zrow")
                    nc.scalar.activation(
                        zrow.rearrange("p (o c) -> p o c", c=8),
                        bass.AP(tensor=zsq.tensor, offset=zsq.offset,
                                ap=[zsq.ap[0], [0, 16], [1, 8]]),
                        mybir.ActivationFunctionType.Copy,
                    )
                    ps_zq = ps_misc.tile([64, 128], f32, tag="m", name="zqps")
                    nc.tensor.matmul(ps_zq, lhsT=onesrow, rhs=zrow,
                                     start=True, stop=True)
                    zqsb = small.tile([64, 128], f32, tag="zqsb", name="zqsb")
                    nc.scalar.activation(zqsb, ps_zq,
                                         mybir.ActivationFunctionType.Copy)

            if it < 2:
                squash_from(fold_n(ps_n), zqsb, True)
                for wd in range(2):
                    nc.tensor.matmul(ps_n[0:64, :], lhsT=ZW[0:64, :],
                                     rhs=Vz[0:64, :], start=False,
                                     stop=False, skip_group_check=True)
            else:
                out_sb = squash_from(fold_n(ps_n), zqsb, False)
                nc.sync.dma_start(out=out_d[:], in_=out_sb)

    nc.finalize()
    return nc


def _host_prep(x, W):
    """Build per-core input dicts."""
    import ml_dtypes
    ct = ml_dtypes.bfloat16
    f8 = ml_dtypes.float8_e4m3fn
    x = np.ascontiguousarray(x, dtype=np.float32)
    W = np.ascontiguousarray(W, dtype=np.float32)
    # xt[p=(q,i), t*64+b] = x[b, 16t+q, i]
    xt = x.reshape(B, NT, 16, I).transpose(2, 3, 1, 0).reshape(128, NT, 64)
    xt8 = np.ascontiguousarray(xt.reshape(128, NT * 64)).astype(f8)
    # xn8p[0:64, hb*512 + u*128 + (q*8+i)] = tile hb*8+u; rows 64:128 get
    # tiles hb*8+4+u (row-pair packing)
    xr = x.reshape(B, NB, 2, 4, 128)
    xn8p = np.concatenate([xr[:, :, 0], xr[:, :, 1]], axis=0)
    xn8p = np.ascontiguousarray(xn8p.reshape(128, NB * 512)).astype(f8)
    cstb, cstf = _consts_np()
    in_maps = []
    for k in range(N_CORES):
        Ws = W[:, k * CL:(k + 1) * CL]  # [R, 8, O, I]
        wkm = (Ws.reshape(NT, 16, CL, O, I).transpose(1, 4, 0, 3, 2)
               .reshape(128, NT, 128))
        im = {
            "xn8p": xn8p,
            "xt8": xt8,
            "xtb": np.ascontiguousarray(xt.reshape(128, NT * 64)).astype(ct),
            "cstb": cstb.astype(ct),
            "cstf": cstf,
        }
        for h in range(8):
            im[f"wk{h}"] = np.ascontiguousarray(
                wkm[:, 16 * h:16 * (h + 1), :].reshape(128, 2048)).astype(ct)
        in_maps.append(im)
    return in_maps


_CACHE = {}


def _get_nc():
    if "nc" not in _CACHE:
        _CACHE["nc"] = build_bass()
    return _CACHE["nc"]


def run(x, W, trace=False):
    nc = _get_nc()
    in_maps = _host_prep(x, W)
    res = run_bass_kernel_spmd(nc, in_maps, core_ids=list(range(N_CORES)),
                               trace=trace)
    outs = [np.asarray(res.results[k]["out"], dtype=np.float32)
            for k in range(N_CORES)]
    # out[b, (o, c)]: core k holds capsules [8k, 8k+8)
    v = np.concatenate(
        [o.reshape(B, O, CL).transpose(0, 2, 1) for o in outs], axis=1)
    return v[..., None], res


def kernel(x, W):
    v, _ = run(np.asarray(x), np.asarray(W))
    return v


# revision 15
# speedup vs baseline: 1.1758x; 1.0451x over previous
"""DigitCapsule dynamic-routing kernel for 8 TRN2 NeuronCores.

Strategy: the reference routing is fully independent per output capsule c
(softmax over routes, sums over routes, batch-mean are all per-c). So we
shard the C=64 capsules 8-ways: each core gets W[:, 8k:8k+8] and a
replicated x. Zero collectives; identical SPMD program per core with
per-core inputs.

Per core (B=64, R=2048, I=8, CL=8, O=16; K-dim = (r,i) = 16384 = 128
k-tiles of 128 = (16 routes q, 8 i)). s/v tensors live as
[b=64, (o,c)=128]; routing state lives banded as [(j,q)=128, (g,lo,c)].

  pass 0:  n0[b,(o,c)] = sum_t xt_t^T @ wk_t    (c_ij uniform), with
           col-paired matmuls (even k-tiles -> psum rows 0:64, odd ->
           64:128; fold with one cross-base add)
           v = n|n| / (R^2 + n^2)       == squash(n/R), exact algebra
  iter 1,2:
    A: G[(q,i),(lo,(o,c))] = xn^T @ V, row-paired: two concurrent K=64
       matmuls (fp8 x halves on partitions 0:64 / 64:128) into the two
       psum banks of each block; per block the P = G (.) Wk multiply
       runs on DVE (direct from psum) or ACT-drain + DVE/GPS
    B: BD-matmul bands psb[(j,q),(lo,o,c)] per grp (interleaved with A's
       G-matmuls so the PE never idles); ored = reduce_o; bstate +=
       ored/B; wexpb = exp(bstate); wrep matmuls merged 4-per-psum-tile
    D: WW = Wr (.) wrep (broadcast o) on DVE/GPS; n += xt_t^T @ WW_t
       col-paired
    Z[c] = sum_r wexp;  v = n|n| / (Z^2 + n^2)  == squash(n/Z), exact
  out[b,(o,c)] = v (f32)

V is duplicated into both partition halves of Vz via a small replication
matmul (I2) so the row-paired G matmuls can read per-half rhs.
"""

import os
import sys

for _p in ("/opt/trn_rl_repo", "/root/.axon_site/_ro/trn_rl_repo"):
    if os.path.isdir(_p) and _p not in sys.path:
        sys.path.insert(0, _p)

from contextlib import ExitStack

import numpy as np

import concourse.bass as bass
import concourse.bacc as bacc
from concourse import mybir
from concourse.bass_utils import run_bass_kernel_spmd
from concourse.tile import TileContext

B, R, C, O, I = 64, 2048, 64, 16, 8
N_CORES = 8
CL = C // N_CORES            # capsules per core = 8
F = CL * O                   # free (o,c) = 128
NT = R // 16                 # 128 k-tiles; tile t = routes [16t,16t+16), part p=(q,i)
NB = 16                      # number of 8-k-tile blocks
BLK = NT // NB               # 8 k-tiles per block

# P-production path per block: DVE-direct from psum / ACT drain + DVE /
# ACT drain + GPS
# GPS tensor ops starve DVE of SBUF bandwidth (measured: a concurrent GPS
# TT slows an identical DVE TT from 690ns to ~2500ns), so all elementwise
# work stays on DVE with ACT doing the PSUM drains.
DIRECT_SET = {0, 5, 9, 13}
P_GPS_SET = set()
WW_GPS_SET = set()


def _consts_np():
    """cstb [128,1152] bf16: BDF4 [0:512), BDT [512:1024), I2 dup [1024:1152).
    cstf [128,65] f32: masked-ones col 0; ones-row (partition 0) cols [1:65)."""
    cstb = np.zeros((128, 1216), dtype=np.float32)
    p = np.arange(128)
    # BDF4_j[p=(q,i), m] = 1 iff m == 32j + p//8  (i-reduce into band 32j+q)
    for j in range(4):
        cstb[p, 128 * j + 32 * j + p // 8] = 1.0
    # BDT_j = BDF4_j^T (band (j,q) -> rows (q,i))
    for j in range(4):
        cstb[:, 512 + 128 * j:512 + 128 * (j + 1)] = \
            cstb[:, 128 * j:128 * (j + 1)].T
    # I2[b, m] = 1 iff m % 64 == b (for b < 64): psum dup of V to both halves
    cstb[p[:64], 1024 + p[:64]] = 1.0
    cstb[p[:64], 1024 + 64 + p[:64]] = 1.0
    cstf = np.zeros((128, 65), dtype=np.float32)
    # Z-reduce mask: only band rows 32j+q (q<16) hold real data; the other
    # 64 partitions of wexpb are exp(0)=1 junk and must not enter Z.
    cstf[p[(p % 32) < 16], 0] = 1.0
    cstf[0, 1:65] = 1.0
    return cstb, cstf


def build_bass():
    f32 = mybir.dt.float32
    cdt = mybir.dt.bfloat16
    f8 = mybir.dt.float8e4

    nc = bacc.Bacc()
    # wk: 8 chunks of 2048 cols of W (bf16), k-tile t at chunk t//16
    wk_d = [nc.declare_dram_parameter(f"wk{h}", [128, 2048], cdt, isOutput=False)
            for h in range(8)]
    # xn8p: fp8 x, pair-packed: rows 0:64 = tiles hb*8+{0..3}, rows 64:128
    # = tiles hb*8+{4..7} (each block hb owns 512 cols)
    xn8p_d = nc.declare_dram_parameter("xn8p", [128, NB * 512], f8, isOutput=False)
    # xt8: fp8 x in (q,i)-partition layout, tile t at cols [64t, 64t+64)
    xt8_d = nc.declare_dram_parameter("xt8", [128, NT * 64], f8, isOutput=False)
    # xtb: bf16 x, same layout; loaded mid-kernel for the final n-pass
    xtb_d = nc.declare_dram_parameter("xtb", [128, NT * 64], cdt, isOutput=False)
    cstb_d = nc.declare_dram_parameter("cstb", [128, 1216], cdt, isOutput=False)
    cstf_d = nc.declare_dram_parameter("cstf", [128, 65], f32, isOutput=False)
    out_d = nc.declare_dram_parameter("out", [B, F], f32, isOutput=True)

    with TileContext(nc) as tc, ExitStack() as ctx:
        big = ctx.enter_context(tc.tile_pool(name="big", bufs=1))
        small = ctx.enter_context(tc.tile_pool(name="small", bufs=3))
        pgpool = ctx.enter_context(tc.tile_pool(name="pgpool", bufs=3))
        p16 = ctx.enter_context(tc.tile_pool(name="p16", bufs=NB + 1))
        wwpool = ctx.enter_context(tc.tile_pool(name="wwpool", bufs=4))
        ps_acc = ctx.enter_context(tc.tile_pool(name="ps_acc", bufs=1, space="PSUM"))
        ps_gb = ctx.enter_context(tc.tile_pool(name="ps_gb", bufs=3, space="PSUM"))
        ps_misc = ctx.enter_context(tc.tile_pool(name="ps_misc", bufs=1, space="PSUM"))

        # ---- load inputs: many small pieces round-robined across the
        # sync/gpsimd/scalar issue queues (per-stream DMA bw is low; the
        # aggregate needs ~20 concurrent streams). Priority order: consts,
        # xt8 + first wk chunks (pass0 critical path), xn8p, rest of wk,
        # then the bf16 xtb which is only needed in iter 2's n-pass.
        cstb = big.tile([128, 1216], cdt, tag="cstb", name="cstb")
        cstf = big.tile([128, 65], f32, tag="cstf", name="cstf")
        xt8 = big.tile([128, NT * 64], f8, tag="xt8", name="xt8")
        wk = [big.tile([128, 2048], cdt, tag=f"wk{h}", name=f"wk{h}")
              for h in range(8)]
        xn8p = big.tile([128, NB * 512], f8, tag="xn8p", name="xn8p")
        xtb = big.tile([128, NT * 64], cdt, tag="xtb", name="xtb")
        dma_q = [nc.sync, nc.gpsimd, nc.scalar, nc.tensor]
        jobs = [(cstb, cstb_d[:]), (cstf, cstf_d[:])]
        for piece in range(4):
            c0 = piece * 2048
            jobs.append((xt8[:, c0:c0 + 2048], xt8_d[:, c0:c0 + 2048]))
        for h in range(8):
            for piece in range(2):
                c0 = piece * 1024
                jobs.append((wk[h][:, c0:c0 + 1024], wk_d[h][:, c0:c0 + 1024]))
        for piece in range(4):
            c0 = piece * 2048
            jobs.append((xn8p[:, c0:c0 + 2048], xn8p_d[:, c0:c0 + 2048]))
        for idx, (dst, srcp) in enumerate(jobs):
            dma_q[idx % 4].dma_start(out=dst, in_=srcp)

        BDF4 = cstb[:, 0:512]
        BDT = cstb[:, 512:1024]
        I2 = cstb[0:64, 1024:1152]
        ZW = cstb[:, 1152:1216]      # zero lhsT for HAM-warm dummy matmuls
        onesm = cstf[:, 0:1]
        onesrow = cstf[0:1, 1:65]

        def wk_tile(t):
            h, lo = t // 16, t % 16
            return wk[h][:, lo * 128:(lo + 1) * 128]

        def wk_block(hb):
            # [128, 8, 128] view of block hb's 8 k-tiles of W
            wkh = wk[hb // 2].rearrange("p (u f) -> p u f", f=128)
            return wkh[:, (hb % 2) * BLK:(hb % 2) * BLK + BLK, :]

        # V: [128,128] bf16; squash writes rows 0:64, dup-matmul fills 64:128
        Vz = big.tile([128, 128], cdt, tag="Vz", name="Vz")

        # v = n*|n| / (zsq + n^2); nf is [64,128] f32 in SBUF
        def squash_from(nf, zsq_sb, mk_V):
            absn = small.tile([64, 128], f32, tag="absn", name="absn")
            nc.scalar.activation(absn, nf, mybir.ActivationFunctionType.Abs)
            nsq = small.tile([64, 128], f32, tag="nsq", name="nsq")
            nc.scalar.activation(nsq, nf, mybir.ActivationFunctionType.Square)
            den = small.tile([64, 128], f32, tag="den", name="den")
            if zsq_sb is None:
                nc.vector.tensor_scalar_add(den, nsq, float(R) * float(R))
            else:
                nc.vector.tensor_add(den, nsq, zsq_sb)
            rden = small.tile([64, 128], f32, tag="rden", name="rden")
            nc.vector.reciprocal_approx_fast(rden, den)
            num = small.tile([64, 128], f32, tag="num", name="num")
            nc.vector.tensor_mul(num, nf, absn)
            if not mk_V:
                out_sb = small.tile([64, 128], f32, tag="outsb", name="outsb")
                nc.vector.tensor_mul(out_sb, num, rden)
                return out_sb
            nc.vector.tensor_mul(Vz[0:64, :], num, rden)
            # duplicate V into rows 64:128 via replication matmul
            ps_dup = ps_misc.tile([128, 128], f32, tag="m", name="dup")
            nc.tensor.matmul(ps_dup, lhsT=I2, rhs=Vz[0:64, :],
                             start=True, stop=True)
            nc.scalar.activation(Vz[64:128, :], ps_dup[64:128, :],
                                 mybir.ActivationFunctionType.Copy)
            return None

        # fold the two col-pair accumulator halves and return n as f32 SBUF
        def fold_n(ps_n):
            nhi = small.tile([64, 128], f32, tag="nhi", name="nhi")
            nc.scalar.activation(nhi, ps_n[64:128, :],
                                 mybir.ActivationFunctionType.Copy)
            nf = small.tile([64, 128], f32, tag="nf", name="nf")
            nc.vector.tensor_add(nf, ps_n[0:64, :], nhi)
            return nf

        # ---- pass 0: n0 = sum_t xt8_t^T @ wk_t (col-paired) ; V = squash ----
        ps_s = ps_acc.tile([128, 128], f32, tag="acc", name="acc")
        for t in range(NT):
            half = t % 2
            nc.tensor.matmul(ps_s[half * 64:(half + 1) * 64, :],
                             lhsT=xt8[:, t * 64:(t + 1) * 64],
                             rhs=wk_tile(t),
                             start=(t < 2), stop=(t >= NT - 2))
        squash_from(fold_n(ps_s), None, True)
        # HAM-warm dummies across the squash gap (zero lhsT accumulates
        # nothing; rhs choices pace them behind pass0 / squash results)
        for wd in range(4):
            nc.tensor.matmul(ps_s[0:64, :], lhsT=ZW,
                             rhs=wk[7][:, wd * 128:(wd + 1) * 128],
                             start=False, stop=False, skip_group_check=True)
        for wd in range(2):
            nc.tensor.matmul(ps_s[0:64, :], lhsT=ZW[0:64, :], rhs=Vz[0:64, :],
                             start=False, stop=False, skip_group_check=True)
        # xtb (bf16 x for iter2's n-pass) loads only now: the issues sit on
        # the scalar queue behind pass0-dependent work, so the transfer does
        # not compete with the critical input phase.
        for piece in range(4):
            c0 = piece * 2048
            nc.scalar.dma_start(out=xtb[:, c0:c0 + 2048],
                                in_=xtb_d[:, c0:c0 + 2048])

        bstate = small.tile([128, 256], f32, tag="bstate", name="bstate", bufs=1)
        nc.vector.memset(bstate, 0.0)
        wexpb = small.tile([128, 256], cdt, tag="wexpb", name="wexpb", bufs=1)

        for it in (1, 2):
            ps_n = ps_acc.tile([128, 128], f32, tag="acc", name="acc")
            nc.tensor.matmul(ps_n[0:64, :], lhsT=ZW[0:64, :], rhs=Vz[0:64, :],
                             start=False, stop=False, skip_group_check=True)
            for wd in range(4):
                nc.tensor.matmul(ps_n[0:64, :], lhsT=ZW,
                                 rhs=wk[wd][:, 0:128],
                                 start=False, stop=False,
                                 skip_group_check=True)
            Ps = [None] * NB
            psbs = [None] * 4
            wrs = [None] * NB

            # -- phase A pieces: G row-pairs + P production for one block --
            def emit_g_block(hb):
                psg = ps_gb.tile([128, BLK * 128], f32, tag="gb", name="gb")
                for u in range(4):
                    cs = slice(hb * 512 + u * 128, hb * 512 + (u + 1) * 128)
                    nc.tensor.matmul(psg[:, u * 128:(u + 1) * 128],
                                     lhsT=xn8p[0:64, cs], rhs=Vz[0:64, :],
                                     start=True, stop=True)
                    nc.tensor.matmul(psg[:, 512 + u * 128:512 + (u + 1) * 128],
                                     lhsT=xn8p[64:128, cs], rhs=Vz[64:128, :],
                                     start=True, stop=True)
                P = p16.tile([128, BLK * 128], cdt, tag="P", name="P")
                if hb in DIRECT_SET:
                    nc.vector.tensor_tensor(
                        P.rearrange("p (u f) -> p u f", f=128),
                        psg.rearrange("p (u f) -> p u f", f=128),
                        wk_block(hb),
                        op=mybir.AluOpType.mult,
                    )
                else:
                    Pg = pgpool.tile([128, BLK * 128], cdt, tag="Pg", name="Pg")
                    nc.scalar.activation(Pg, psg,
                                         mybir.ActivationFunctionType.Copy)
                    eng = nc.gpsimd if hb in P_GPS_SET else nc.vector
                    eng.tensor_tensor(
                        P.rearrange("p (u f) -> p u f", f=128),
                        Pg.rearrange("p (u f) -> p u f", f=128),
                        wk_block(hb),
                        op=mybir.AluOpType.mult,
                    )
                Ps[hb] = P
                if hb >= 2:
                    nc.tensor.matmul(ps_n[0:64, :], lhsT=ZW,
                                     rhs=Ps[hb - 2][:, 0:128],
                                     start=False, stop=False,
                                     skip_group_check=True)

            # -- phase B pieces --
            def emit_bd(grp):
                psb = ps_gb.tile([128, BLK * 128], f32, tag="gb", name="gb")
                for j in range(4):
                    for half in range(2):
                        nc.tensor.matmul(
                            psb[:, half * 512:(half + 1) * 512],
                            lhsT=BDF4[:, 128 * j:128 * (j + 1)],
                            rhs=Ps[4 * grp + j][:, half * 512:(half + 1) * 512],
                            start=(j == 0), stop=(j == 3),
                        )
                psbs[grp] = psb

            def emit_bupdate_dve(grp):
                ored = small.tile([128, 64], f32, tag="ored", name="ored",
                                  bufs=2)
                psb = psbs[grp]
                nc.vector.tensor_reduce(
                    ored.rearrange("p (l c) -> p l c", c=8),
                    bass.AP(tensor=psb.tensor, offset=psb.offset,
                            ap=[psb.ap[0], [128, 8], [1, 8], [8, 16]]),
                    axis=mybir.AxisListType.X,
                    op=mybir.AluOpType.add,
                )
                cs = slice(grp * 64, (grp + 1) * 64)
                nc.vector.scalar_tensor_tensor(bstate[:, cs], ored, 1.0 / B,
                                               bstate[:, cs],
                                               op0=mybir.AluOpType.mult,
                                               op1=mybir.AluOpType.add)

            def emit_bupdate_act(grp):
                cs = slice(grp * 64, (grp + 1) * 64)
                nc.scalar.activation(wexpb[:, cs], bstate[:, cs],
                                     mybir.ActivationFunctionType.Exp)

            def emit_wrep(grp):
                cs = slice(grp * 64, (grp + 1) * 64)
                ps_wr = ps_misc.tile([128, 256], f32, tag="m", name="wrps")
                for j in range(4):
                    nc.tensor.matmul(ps_wr[:, j * 64:(j + 1) * 64],
                                     lhsT=BDT[:, 128 * j:128 * (j + 1)],
                                     rhs=wexpb[:, cs], start=True, stop=True)
                wr4 = small.tile([128, 256], cdt, tag="wr", name="wr", bufs=2)
                nc.scalar.activation(wr4, ps_wr,
                                     mybir.ActivationFunctionType.Copy)
                for j in range(4):
                    wrs[4 * grp + j] = wr4[:, j * 64:(j + 1) * 64]

            # slotted interleave of A (G blocks), B (BD/update/wrep) and
            # D (WW + n matmuls): every op is emitted onto its engine queue
            # in expected data-ready order, so the in-order queues never
            # head-block (an early exp/wr4 on the scalar queue would stall
            # all later G drains behind it)
            emit_g_block(0)
            emit_g_block(1)
            emit_g_block(2)
            emit_g_block(3)
            emit_bd(0)
            emit_g_block(4)
            emit_g_block(5)
            emit_g_block(6)
            emit_bupdate_dve(0)
            emit_g_block(7)
            emit_bupdate_act(0)
            emit_wrep(0)
            emit_bd(1)
            emit_g_block(8)
            emit_ww_n(0)
            emit_g_block(9)
            emit_ww_n(1)
            emit_g_block(10)
            emit_bupdate_dve(1)
            emit_g_block(11)
            emit_bupdate_act(1)
            emit_wrep(1)
            emit_bd(2)
            emit_g_block(12)
            emit_ww_n(2)
            emit_g_block(13)
            emit_ww_n(3)
            emit_g_block(14)
            emit_bupdate_dve(2)
            emit_g_block(15)
            emit_bupdate_act(2)
            emit_wrep(2)
            emit_bd(3)
            emit_ww_n(4)
            emit_ww_n(5)
            emit_ww_n(6)
            emit_bupdate_dve(3)
            emit_bupdate_act(3)
            emit_wrep(3)
            emit_ww_n(7)
            emit_ww_n(8)
            emit_ww_n(9)
            emit_ww_n(10)
            emit_ww_n(11)
            emit_ww_n(12)
            emit_ww_n(13)
            zqsb = None
            for hb in range(14, NB):
                emit_ww_n(hb)
                if hb == 14:
                    # Z^2 per c, replicated to [64, 128] (overlaps phase D;
                    # emitted after two WW blocks so it doesn't head-block
                    # the vector queue)
                    wsum = small.tile([128, 8], f32, tag="wsum", name="wsum")
                    nc.vector.tensor_reduce(
                        wsum,
                        bass.AP(tensor=wexpb.tensor, offset=wexpb.offset,
                                ap=[wexpb.ap[0], [1, 8], [8, 32]]),
                        axis=mybir.AxisListType.X, op=mybir.AluOpType.add,
                    )
                    ps_z = ps_misc.tile([1, 8], f32, tag="m", name="zps")
                    nc.tensor.matmul(ps_z, lhsT=onesm, rhs=wsum,
                                     start=True, stop=True)
                    zsq = small.tile([1, 8], f32, tag="zsq", name="zsq")
                    nc.scalar.activation(zsq, ps_z,
                                         mybir.ActivationFunctionType.Square)
                    zrow = small.tile([1, 128], f32, tag="zrow", name="zrow")
                    nc.scalar.activation(
                        zrow.rearrange("p (o c) -> p o c", c=8),
                        bass.AP(tensor=zsq.tensor, offset=zsq.offset,
                                ap=[zsq.ap[0], [0, 16], [1, 8]]),
                        mybir.ActivationFunctionType.Copy,
                    )
                    ps_zq = ps_misc.tile([64, 128], f32, tag="m", name="zqps")
                    nc.tensor.matmul(ps_zq, lhsT=onesrow, rhs=zrow,
                                     start=True, stop=True)
                    zqsb = small.tile([64, 128], f32, tag="zqsb", name="zqsb")
                    nc.scalar.activation(zqsb, ps_zq,
                                         mybir.ActivationFunctionType.Copy)

            if it < 2:
                squash_from(fold_n(ps_n), zqsb, True)
                for wd in range(2):
                    nc.tensor.matmul(ps_n[0:64, :], lhsT=ZW[0:64, :],
                                     rhs=Vz[0:64, :], start=False,
                                     stop=False, skip_group_check=True)
            else:
                out_sb = squash_from(fold_n(ps_n), zqsb, False)
                nc.sync.dma_start(out=out_d[:], in_=out_sb)

    nc.finalize()
    return nc


def _host_prep(x, W):
    """Build per-core input dicts."""
    import ml_dtypes
    ct = ml_dtypes.bfloat16
    f8 = ml_dtypes.float8_e4m3fn
    x = np.ascontiguousarray(x, dtype=np.float32)
    W = np.ascontiguousarray(W, dtype=np.float32)
    # xt[p=(q,i), t*64+b] = x[b, 16t+q, i]
    xt = x.reshape(B, NT, 16, I).transpose(2, 3, 1, 0).reshape(128, NT, 64)
    xt8 = np.ascontiguousarray(xt.reshape(128, NT * 64)).astype(f8)
    # xn8p[0:64, hb*512 + u*128 + (q*8+i)] = tile hb*8+u; rows 64:128 get
    # tiles hb*8+4+u (row-pair packing)
    xr = x.reshape(B, NB, 2, 4, 128)
    xn8p = np.concatenate([xr[:, :, 0], xr[:, :, 1]], axis=0)
    xn8p = np.ascontiguousarray(xn8p.reshape(128, NB * 512)).astype(f8)
    cstb, cstf = _consts_np()
    in_maps = []
    for k in range(N_CORES):
        Ws = W[:, k * CL:(k + 1) * CL]  # [R, 8, O, I]
        wkm = (Ws.reshape(NT, 16, CL, O, I).transpose(1, 4, 0, 3, 2)
               .reshape(128, NT, 128))
        im = {
            "xn8p": xn8p,
            "xt8": xt8,
            "xtb": np.ascontiguousarray(xt.reshape(128, NT * 64)).astype(ct),
            "cstb": cstb.astype(ct),
            "cstf": cstf,
        }
        for h in range(8):
            im[f"wk{h}"] = np.ascontiguousarray(
                wkm[:, 16 * h:16 * (h + 1), :].reshape(128, 2048)).astype(ct)
        in_maps.append(im)
    return in_maps


_CACHE = {}


def _get_nc():
    if "nc" not in _CACHE:
        _CACHE["nc"] = build_bass()
    return _CACHE["nc"]


def run(x, W, trace=False):
    nc = _get_nc()
    in_maps = _host_prep(x, W)
    res = run_bass_kernel_spmd(nc, in_maps, core_ids=list(range(N_CORES)),
                               trace=trace)
    outs = [np.asarray(res.results[k]["out"], dtype=np.float32)
            for k in range(N_CORES)]
    # out[b, (o, c)]: core k holds capsules [8k, 8k+8)
    v = np.concatenate(
        [o.reshape(B, O, CL).transpose(0, 2, 1) for o in outs], axis=1)
    return v[..., None], res


def kernel(x, W):
    v, _ = run(np.asarray(x), np.asarray(W))
    return v
